# revision 15
# baseline (speedup 1.0000x reference)
"""Trainium2 Bass kernel for nn_CausalAttentionPooling.

Math: scores[b,i,j] = x[b,i].q are constant along the softmax axis j, so
softmax over the causal mask yields uniform weights 1/(i+1) on j <= i.
The module is exactly a causal cumulative mean:
    out[b,i,:] = cumsum(x, axis=1)[b,i,:] / (i+1)
(q does not affect the output.)

v2 design (fp16 I/O, quad-decimated scan, DVE-only math):
  - 8 shards = (batch b) x (D-half dh); per core xT = x[b,:,dh*128:+128].T
    as [128(D), 4096(L)] cast to fp16 on host (tolerance is 2e-2; fp16
    roundtrip error ~5e-4).
  - Host de-interleaves L into quad phases, chunk-major:
    X[:, c*2048 + p*512 + k] = xT[:, 4*(c*512+k)+p].  The DVE scan (which
    has no 2x mode and runs at ~2.1 ns/elem) then only processes the
    pairwise-sum sequence y_k = sum_p x_{4k+p} (1024 elems instead of
    4096); everything else runs as 2x-mode fp16 tensor_tensor ops
    (~0.56 ns/elem):
      u = A+B; v = C+D; y = u+v
      cum = scan(y)            -> written into the D slot of S
      s_c = cum - D; s_b = s_c - C; s_a = s_b - B   (chained subs)
      OUT = S * RR             (one batched multiply per chunk)
  - RR = 1/(i+1) in the same chunk-major quad layout, shipped from HBM
    quarter-replicated [32, 4096] and partition-replicated on-chip by
    three SBUF->SBUF DMAs (no PE broadcast, no PSUM).
  - Scan accumulates in fp32 internally regardless of operand dtype.
  - Two chunks (c=0,1) pipeline DMA-in / DVE / DMA-out; chunk 1's scan
    chains via initial=S[:, 2047:2048].
"""

import numpy as np

B, L, D = 4, 4096, 256
NCORES = 8
P = 128            # partitions / D-shard width
NCH = 2            # chunks along L
CW = L // NCH      # columns per chunk (2048)
PW = CW // 4       # phase width within a chunk (512)

_cache = {}


def _split_waits_bir(bir_bytes):
    """This container's walrus build rejects instructions carrying more than
    one (or for some opcodes, two) sync waits.  Hoist multi-wait sync_info
    onto standalone same-engine EventSemaphore instructions inserted
    immediately before the instruction; program order on the engine's stream
    preserves semantics."""
    import orjson

    d = orjson.loads(bir_bytes)
    n = 0
    for fn in d["functions"]:
        for bb in fn["blocks"]:
            out = []
            for inst in bb["instructions"]:
                si = inst.get("sync_info")
                waits = (si or {}).get("on_wait") or []
                if len(waits) > 1:
                    for w in waits:
                        out.append(
                            {
                                "debug": inst.get("debug"),
                                "engine": inst["engine"],
                                "ins": [],
                                "name": f"I-waitfix-{n}",
                                "opcode": "EventSemaphore",
                                "outs": [],
                                "sync_info": {"on_wait": [w], "on_update": []},
                            }
                        )
                        n += 1
                    si["on_wait"] = []
                out.append(inst)
            bb["instructions"] = out
    return orjson.dumps(d)


def _install_bir_patch():
    if _cache.get("patched"):
        return
    import concourse.bass as bass

    orig = bass.Bass.to_json_bytes

    def patched(self):
        return _split_waits_bir(orig(self))

    bass.Bass.to_json_bytes = patched
    _cache["patched"] = True


def _build_nc():
    import concourse.bass as bass
    import concourse.tile as tile
    from concourse import mybir

    _install_bir_patch()

    f16 = mybir.dt.float16
    add = mybir.AluOpType.add
    sub = mybir.AluOpType.subtract
    byp = mybir.AluOpType.bypass
    mult = mybir.AluOpType.mult

    nc = bass.Bass()
    X = nc.declare_dram_parameter("X", [P, L], f16, isOutput=False)
    RRF = nc.declare_dram_parameter("RRF", [P, L], f16, isOutput=False)
    OUT = nc.declare_dram_parameter("OUT", [P, L], f16, isOutput=True)

    with tile.TileContext(nc) as tc:
        with tc.tile_pool(name="sb", bufs=1) as sb:
            xt = sb.tile([P, L], f16, tag="xt")
            rr = sb.tile([P, L], f16, tag="rr")
            S = sb.tile([P, L], f16, tag="S")
            ot = sb.tile([P, L], f16, tag="ot")
            uv = [
                (sb.tile([P, PW], f16, tag=f"u{c}", name=f"u{c}"),
                 sb.tile([P, PW], f16, tag=f"v{c}", name=f"v{c}"))
                for c in range(NCH)
            ]

            # input DMAs: chunk0's tree operands split pairwise across the
            # Sync and Scalar queues so each lands as early as possible
            # (concurrent DMAs round-robin at packet level, so one big DMA
            # finishes late); RRQ + its partition-replication (3 SBUF->SBUF
            # copies) trail on the Scalar queue -- rr is only needed by the
            # first OUT multiply, several microseconds later.
            # rr ships fully replicated from HBM (+1MB read) -- cheaper in
            # latency than on-chip partition replication, whose 2-hop
            # HBM->SBUF->SBUF chain kept gating the first OUT multiply.
            # both chunk0 halves ride the Sync queue back-to-back (the
            # Scalar queue's first transfer starts ~1us later); chunk 1 and
            # rr overlap on the Scalar queue
            nc.sync.dma_start(xt[:, 0:1024], X[:, 0:1024])          # A0 B0
            nc.sync.dma_start(xt[:, 1024:2048], X[:, 1024:2048])    # C0 D0
            nc.scalar.dma_start(xt[:, 2048:4096], X[:, 2048:4096])  # chunk 1
            nc.scalar.dma_start(rr[:, 0:2048], RRF[:, 0:2048])
            nc.scalar.dma_start(rr[:, 2048:4096], RRF[:, 2048:4096])

            for c in range(NCH):
                o = c * CW          # chunk column offset
                a, b_, cc, dd = o, o + PW, o + 2 * PW, o + 3 * PW
                u, v = uv[c]
                nc.vector.tensor_tensor(u[:], xt[:, a:a + PW], xt[:, b_:b_ + PW], op=add)
                nc.vector.tensor_tensor(v[:], xt[:, cc:cc + PW], xt[:, dd:dd + PW], op=add)
                init = 0.0 if c == 0 else S[:, o - 1:o]
                # cum -> D slot of S; the scan's op1 folds in the final tree
                # add: state = (u_t + state) + v_t
                nc.vector.tensor_tensor_scan(
                    S[:, dd:dd + PW], u[:], v[:], init, op0=add, op1=add
                )
                # chained reconstruction of the other three phase sums
                nc.vector.tensor_tensor(
                    S[:, cc:cc + PW], S[:, dd:dd + PW], xt[:, dd:dd + PW], op=sub
                )
                nc.vector.tensor_tensor(
                    S[:, b_:b_ + PW], S[:, cc:cc + PW], xt[:, cc:cc + PW], op=sub
                )
                nc.vector.tensor_tensor(
                    S[:, a:a + PW], S[:, b_:b_ + PW], xt[:, b_:b_ + PW], op=sub
                )
                # batched scale via 2x tensor_tensor; output DMAs go on the
                # Scalar queue (idle after C0D0).  The last chunk's multiply
                # is split so its first piece's DMA starts earlier and the
                # final transfer (and thus the completion tail) is short.
                if c < NCH - 1:
                    nc.vector.tensor_tensor(
                        ot[:, o:o + CW], S[:, o:o + CW], rr[:, o:o + CW], op=mult
                    )
                    nc.scalar.dma_start(OUT[:, o:o + CW], ot[:, o:o + CW])
                else:
                    m = o + 1536
                    nc.vector.tensor_tensor(
                        ot[:, o:m], S[:, o:m], rr[:, o:m], op=mult
                    )
                    # split the final transfer across both queues so the
                    # completion tail is half as long
                    h = o + 768
                    nc.sync.dma_start(OUT[:, o:h], ot[:, o:h])
                    nc.scalar.dma_start(OUT[:, h:m], ot[:, h:m])
                    nc.vector.tensor_tensor(
                        ot[:, m:o + CW], S[:, m:o + CW], rr[:, m:o + CW], op=mult
                    )
                    nc.scalar.dma_start(OUT[:, m:o + CW], ot[:, m:o + CW])
    return nc


def _get_nc():
    if "nc" not in _cache:
        _cache["nc"] = _build_nc()
    return _cache["nc"]


def _quad_layout(rowmajor):
    """[rows, L] -> chunk-major quad de-interleave along the last axis:
    out[:, c*2048 + p*512 + k] = in[:, 4*(c*512+k)+p]"""
    r = rowmajor.reshape(-1, NCH, PW, 4)         # [rows, c, k, p]
    return np.ascontiguousarray(r.transpose(0, 1, 3, 2)).reshape(-1, L)


def _quad_unlayout(quad):
    """inverse of _quad_layout"""
    r = quad.reshape(-1, NCH, 4, PW)             # [rows, c, p, k]
    return np.ascontiguousarray(r.transpose(0, 1, 3, 2)).reshape(-1, L)


def _make_in_maps(x):
    idx = np.arange(1, L + 1, dtype=np.float64)
    rrow = (1.0 / idx).astype(np.float16).reshape(1, L)
    rrf = np.ascontiguousarray(np.broadcast_to(_quad_layout(rrow), (P, L)))
    in_maps = []
    shards = []
    for c in range(NCORES):
        b, dh = c // 2, c % 2
        shards.append((b, dh))
        xT = np.ascontiguousarray(x[b, :, dh * P:(dh + 1) * P].T).astype(np.float16)
        in_maps.append({"X": _quad_layout(xT), "RRF": rrf})
    return in_maps, shards


def kernel(x, q):
    from concourse.bass_utils import run_bass_kernel_spmd

    x = np.asarray(x)
    assert x.shape == (B, L, D) and x.dtype == np.float32

    nc = _get_nc()
    in_maps, shards = _make_in_maps(x)
    results = run_bass_kernel_spmd(nc, in_maps, list(range(NCORES))).results

    out = np.empty((B, L, D), dtype=np.float32)
    for c, (b, dh) in enumerate(shards):
        outT = _quad_unlayout(results[c]["OUT"]).astype(np.float32)
        out[b, :, dh * P:(dh + 1) * P] = outT.T
    return out


# revision 16
# speedup vs baseline: 1.0050x; 1.0050x over previous
"""Trainium2 Bass kernel for nn_CausalAttentionPooling.

Math: scores[b,i,j] = x[b,i].q are constant along the softmax axis j, so
softmax over the causal mask yields uniform weights 1/(i+1) on j <= i.
The module is exactly a causal cumulative mean:
    out[b,i,:] = cumsum(x, axis=1)[b,i,:] / (i+1)
(q does not affect the output.)

v2 design (fp16 I/O, quad-decimated scan, DVE-only math):
  - 8 shards = (batch b) x (D-half dh); per core xT = x[b,:,dh*128:+128].T
    as [128(D), 4096(L)] cast to fp16 on host (tolerance is 2e-2; fp16
    roundtrip error ~5e-4).
  - Host de-interleaves L into quad phases, chunk-major:
    X[:, c*2048 + p*512 + k] = xT[:, 4*(c*512+k)+p].  The DVE scan (which
    has no 2x mode and runs at ~2.1 ns/elem) then only processes the
    pairwise-sum sequence y_k = sum_p x_{4k+p} (1024 elems instead of
    4096); everything else runs as 2x-mode fp16 tensor_tensor ops
    (~0.56 ns/elem):
      u = A+B; v = C+D; y = u+v
      cum = scan(y)            -> written into the D slot of S
      s_c = cum - D; s_b = s_c - C; s_a = s_b - B   (chained subs)
      OUT = S * RR             (one batched multiply per chunk)
  - RR = 1/(i+1) in the same chunk-major quad layout, shipped from HBM
    quarter-replicated [32, 4096] and partition-replicated on-chip by
    three SBUF->SBUF DMAs (no PE broadcast, no PSUM).
  - Scan accumulates in fp32 internally regardless of operand dtype.
  - Two chunks (c=0,1) pipeline DMA-in / DVE / DMA-out; chunk 1's scan
    chains via initial=S[:, 2047:2048].
"""

import numpy as np

B, L, D = 4, 4096, 256
NCORES = 8
P = 128            # partitions / D-shard width
NCH = 2            # chunks along L
CW = L // NCH      # columns per chunk (2048)
PW = CW // 4       # phase width within a chunk (512)

_cache = {}


def _split_waits_bir(bir_bytes):
    """This container's walrus build rejects instructions carrying more than
    one (or for some opcodes, two) sync waits.  Hoist multi-wait sync_info
    onto standalone same-engine EventSemaphore instructions inserted
    immediately before the instruction; program order on the engine's stream
    preserves semantics."""
    import orjson

    d = orjson.loads(bir_bytes)
    n = 0
    for fn in d["functions"]:
        for bb in fn["blocks"]:
            out = []
            for inst in bb["instructions"]:
                si = inst.get("sync_info")
                waits = (si or {}).get("on_wait") or []
                if len(waits) > 1:
                    for w in waits:
                        out.append(
                            {
                                "debug": inst.get("debug"),
                                "engine": inst["engine"],
                                "ins": [],
                                "name": f"I-waitfix-{n}",
                                "opcode": "EventSemaphore",
                                "outs": [],
                                "sync_info": {"on_wait": [w], "on_update": []},
                            }
                        )
                        n += 1
                    si["on_wait"] = []
                out.append(inst)
            bb["instructions"] = out
    return orjson.dumps(d)


def _install_bir_patch():
    if _cache.get("patched"):
        return
    import concourse.bass as bass

    orig = bass.Bass.to_json_bytes

    def patched(self):
        return _split_waits_bir(orig(self))

    bass.Bass.to_json_bytes = patched
    _cache["patched"] = True


def _build_nc():
    import concourse.bass as bass
    import concourse.tile as tile
    from concourse import mybir

    _install_bir_patch()

    f16 = mybir.dt.float16
    add = mybir.AluOpType.add
    sub = mybir.AluOpType.subtract
    byp = mybir.AluOpType.bypass
    mult = mybir.AluOpType.mult

    nc = bass.Bass()
    X = nc.declare_dram_parameter("X", [P, L], f16, isOutput=False)
    RRF = nc.declare_dram_parameter("RRF", [P, L], f16, isOutput=False)
    OUT = nc.declare_dram_parameter("OUT", [P, L], f16, isOutput=True)

    with tile.TileContext(nc) as tc:
        with tc.tile_pool(name="sb", bufs=1) as sb:
            xt = sb.tile([P, L], f16, tag="xt")
            rr = sb.tile([P, L], f16, tag="rr")
            S = sb.tile([P, L], f16, tag="S")
            ot = sb.tile([P, L], f16, tag="ot")
            uv = [
                (sb.tile([P, PW], f16, tag=f"u{c}", name=f"u{c}"),
                 sb.tile([P, PW], f16, tag=f"v{c}", name=f"v{c}"))
                for c in range(NCH)
            ]

            # input DMAs: chunk0's tree operands split pairwise across the
            # Sync and Scalar queues so each lands as early as possible
            # (concurrent DMAs round-robin at packet level, so one big DMA
            # finishes late); RRQ + its partition-replication (3 SBUF->SBUF
            # copies) trail on the Scalar queue -- rr is only needed by the
            # first OUT multiply, several microseconds later.
            # rr ships fully replicated from HBM (+1MB read) -- cheaper in
            # latency than on-chip partition replication, whose 2-hop
            # HBM->SBUF->SBUF chain kept gating the first OUT multiply.
            # both chunk0 halves ride the Sync queue back-to-back (the
            # Scalar queue's first transfer starts ~1us later); chunk 1 and
            # rr overlap on the Scalar queue
            nc.sync.dma_start(xt[:, 0:1024], X[:, 0:1024])          # A0 B0
            nc.sync.dma_start(xt[:, 1024:2048], X[:, 1024:2048])    # C0 D0
            nc.sync.dma_start(xt[:, 2048:4096], X[:, 2048:4096])    # chunk 1
            nc.scalar.dma_start(rr[:, 0:2048], RRF[:, 0:2048])
            nc.scalar.dma_start(rr[:, 2048:4096], RRF[:, 2048:4096])

            for c in range(NCH):
                o = c * CW          # chunk column offset
                a, b_, cc, dd = o, o + PW, o + 2 * PW, o + 3 * PW
                u, v = uv[c]
                nc.vector.tensor_tensor(u[:], xt[:, a:a + PW], xt[:, b_:b_ + PW], op=add)
                nc.vector.tensor_tensor(v[:], xt[:, cc:cc + PW], xt[:, dd:dd + PW], op=add)
                init = 0.0 if c == 0 else S[:, o - 1:o]
                # cum -> D slot of S; the scan's op1 folds in the final tree
                # add: state = (u_t + state) + v_t
                nc.vector.tensor_tensor_scan(
                    S[:, dd:dd + PW], u[:], v[:], init, op0=add, op1=add
                )
                # chained reconstruction of the other three phase sums
                nc.vector.tensor_tensor(
                    S[:, cc:cc + PW], S[:, dd:dd + PW], xt[:, dd:dd + PW], op=sub
                )
                nc.vector.tensor_tensor(
                    S[:, b_:b_ + PW], S[:, cc:cc + PW], xt[:, cc:cc + PW], op=sub
                )
                nc.vector.tensor_tensor(
                    S[:, a:a + PW], S[:, b_:b_ + PW], xt[:, b_:b_ + PW], op=sub
                )
                # batched scale via 2x tensor_tensor; output DMAs go on the
                # Scalar queue (idle after C0D0).  The last chunk's multiply
                # is split so its first piece's DMA starts earlier and the
                # final transfer (and thus the completion tail) is short.
                if c < NCH - 1:
                    nc.vector.tensor_tensor(
                        ot[:, o:o + CW], S[:, o:o + CW], rr[:, o:o + CW], op=mult
                    )
                    nc.scalar.dma_start(OUT[:, o:o + CW], ot[:, o:o + CW])
                else:
                    m = o + 1536
                    nc.vector.tensor_tensor(
                        ot[:, o:m], S[:, o:m], rr[:, o:m], op=mult
                    )
                    # split the final transfer across both queues so the
                    # completion tail is half as long
                    h = o + 768
                    nc.sync.dma_start(OUT[:, o:h], ot[:, o:h])
                    nc.scalar.dma_start(OUT[:, h:m], ot[:, h:m])
                    nc.vector.tensor_tensor(
                        ot[:, m:o + CW], S[:, m:o + CW], rr[:, m:o + CW], op=mult
                    )
                    nc.scalar.dma_start(OUT[:, m:o + CW], ot[:, m:o + CW])
    return nc


def _get_nc():
    if "nc" not in _cache:
        _cache["nc"] = _build_nc()
    return _cache["nc"]


def _quad_layout(rowmajor):
    """[rows, L] -> chunk-major quad de-interleave along the last axis:
    out[:, c*2048 + p*512 + k] = in[:, 4*(c*512+k)+p]"""
    r = rowmajor.reshape(-1, NCH, PW, 4)         # [rows, c, k, p]
    return np.ascontiguousarray(r.transpose(0, 1, 3, 2)).reshape(-1, L)


def _quad_unlayout(quad):
    """inverse of _quad_layout"""
    r = quad.reshape(-1, NCH, 4, PW)             # [rows, c, p, k]
    return np.ascontiguousarray(r.transpose(0, 1, 3, 2)).reshape(-1, L)


def _make_in_maps(x):
    idx = np.arange(1, L + 1, dtype=np.float64)
    rrow = (1.0 / idx).astype(np.float16).reshape(1, L)
    rrf = np.ascontiguousarray(np.broadcast_to(_quad_layout(rrow), (P, L)))
    in_maps = []
    shards = []
    for c in range(NCORES):
        b, dh = c // 2, c % 2
        shards.append((b, dh))
        xT = np.ascontiguousarray(x[b, :, dh * P:(dh + 1) * P].T).astype(np.float16)
        in_maps.append({"X": _quad_layout(xT), "RRF": rrf})
    return in_maps, shards


def kernel(x, q):
    from concourse.bass_utils import run_bass_kernel_spmd

    x = np.asarray(x)
    assert x.shape == (B, L, D) and x.dtype == np.float32

    nc = _get_nc()
    in_maps, shards = _make_in_maps(x)
    results = run_bass_kernel_spmd(nc, in_maps, list(range(NCORES))).results

    out = np.empty((B, L, D), dtype=np.float32)
    for c, (b, dh) in enumerate(shards):
        outT = _quad_unlayout(results[c]["OUT"]).astype(np.float32)
        out[b, :, dh * P:(dh + 1) * P] = outT.T
    return out


# revision 17
# speedup vs baseline: 1.0987x; 1.0933x over previous
"""Trainium2 Bass kernel for nn_CausalAttentionPooling.

Math: scores[b,i,j] = x[b,i].q are constant along the softmax axis j, so
softmax over the causal mask yields uniform weights 1/(i+1) on j <= i.
The module is exactly a causal cumulative mean:
    out[b,i,:] = cumsum(x, axis=1)[b,i,:] / (i+1)
(q does not affect the output.)

v2 design (fp16 I/O, quad-decimated scan, DVE-only math):
  - 8 shards = (batch b) x (D-half dh); per core xT = x[b,:,dh*128:+128].T
    as [128(D), 4096(L)] cast to fp16 on host (tolerance is 2e-2; fp16
    roundtrip error ~5e-4).
  - Host de-interleaves L into quad phases, chunk-major:
    X[:, c*2048 + p*512 + k] = xT[:, 4*(c*512+k)+p].  The DVE scan (which
    has no 2x mode and runs at ~2.1 ns/elem) then only processes the
    pairwise-sum sequence y_k = sum_p x_{4k+p} (1024 elems instead of
    4096); everything else runs as 2x-mode fp16 tensor_tensor ops
    (~0.56 ns/elem):
      u = A+B; v = C+D; y = u+v
      cum = scan(y)            -> written into the D slot of S
      s_c = cum - D; s_b = s_c - C; s_a = s_b - B   (chained subs)
      OUT = S * RR             (one batched multiply per chunk)
  - RR = 1/(i+1) in the same chunk-major quad layout, shipped from HBM
    quarter-replicated [32, 4096] and partition-replicated on-chip by
    three SBUF->SBUF DMAs (no PE broadcast, no PSUM).
  - Scan accumulates in fp32 internally regardless of operand dtype.
  - Two chunks (c=0,1) pipeline DMA-in / DVE / DMA-out; chunk 1's scan
    chains via initial=S[:, 2047:2048].
"""

import numpy as np

B, L, D = 4, 4096, 256
NCORES = 8
P = 128            # partitions / D-shard width
NCH = 2            # chunks along L
CW = L // NCH      # columns per chunk (2048)
PW = CW // 4       # phase width within a chunk (512)

_cache = {}


def _split_waits_bir(bir_bytes):
    """This container's walrus build rejects instructions carrying more than
    one (or for some opcodes, two) sync waits.  Hoist multi-wait sync_info
    onto standalone same-engine EventSemaphore instructions inserted
    immediately before the instruction; program order on the engine's stream
    preserves semantics."""
    import orjson

    d = orjson.loads(bir_bytes)
    n = 0
    for fn in d["functions"]:
        for bb in fn["blocks"]:
            out = []
            for inst in bb["instructions"]:
                si = inst.get("sync_info")
                waits = (si or {}).get("on_wait") or []
                if len(waits) > 1:
                    for w in waits:
                        out.append(
                            {
                                "debug": inst.get("debug"),
                                "engine": inst["engine"],
                                "ins": [],
                                "name": f"I-waitfix-{n}",
                                "opcode": "EventSemaphore",
                                "outs": [],
                                "sync_info": {"on_wait": [w], "on_update": []},
                            }
                        )
                        n += 1
                    si["on_wait"] = []
                out.append(inst)
            bb["instructions"] = out
    return orjson.dumps(d)


def _install_bir_patch():
    if _cache.get("patched"):
        return
    import concourse.bass as bass

    orig = bass.Bass.to_json_bytes

    def patched(self):
        return _split_waits_bir(orig(self))

    bass.Bass.to_json_bytes = patched
    _cache["patched"] = True


def _build_nc():
    import concourse.bass as bass
    import concourse.tile as tile
    from concourse import mybir

    _install_bir_patch()

    f16 = mybir.dt.float16
    add = mybir.AluOpType.add
    sub = mybir.AluOpType.subtract
    byp = mybir.AluOpType.bypass
    mult = mybir.AluOpType.mult

    nc = bass.Bass()
    X = nc.declare_dram_parameter("X", [P, L], f16, isOutput=False)
    RRF = nc.declare_dram_parameter("RRF", [P, L], f16, isOutput=False)
    OUT = nc.declare_dram_parameter("OUT", [P, L], f16, isOutput=True)

    with tile.TileContext(nc) as tc:
        with tc.tile_pool(name="sb", bufs=1) as sb:
            xt = sb.tile([P, L], f16, tag="xt")
            rr = sb.tile([P, L], f16, tag="rr")
            S = sb.tile([P, L], f16, tag="S")
            ot = sb.tile([P, L], f16, tag="ot")
            uv = [
                (sb.tile([P, PW], f16, tag=f"u{c}", name=f"u{c}"),
                 sb.tile([P, PW], f16, tag=f"v{c}", name=f"v{c}"))
                for c in range(NCH)
            ]

            # input DMAs: chunk0's tree operands split pairwise across the
            # Sync and Scalar queues so each lands as early as possible
            # (concurrent DMAs round-robin at packet level, so one big DMA
            # finishes late); RRQ + its partition-replication (3 SBUF->SBUF
            # copies) trail on the Scalar queue -- rr is only needed by the
            # first OUT multiply, several microseconds later.
            # rr ships fully replicated from HBM (+1MB read) -- cheaper in
            # latency than on-chip partition replication, whose 2-hop
            # HBM->SBUF->SBUF chain kept gating the first OUT multiply.
            # both chunk0 halves ride the Sync queue back-to-back (the
            # Scalar queue's first transfer starts ~1us later); chunk 1 and
            # rr overlap on the Scalar queue
            # single need-ordered input stream on the Sync queue: each piece
            # lands just before its consumer, and nothing steals HBM
            # bandwidth from the critical path (concurrent queues round-robin
            # at packet granularity).  Outputs get the Scalar queue.
            nc.sync.dma_start(xt[:, 0:1024], X[:, 0:1024])          # A0 B0
            nc.sync.dma_start(xt[:, 1024:2048], X[:, 1024:2048])    # C0 D0
            nc.sync.dma_start(xt[:, 2048:3072], X[:, 2048:3072])    # A1 B1
            nc.sync.dma_start(xt[:, 3072:4096], X[:, 3072:4096])    # C1 D1
            nc.sync.dma_start(rr[:, 0:2048], RRF[:, 0:2048])
            nc.sync.dma_start(rr[:, 2048:4096], RRF[:, 2048:4096])

            for c in range(NCH):
                o = c * CW          # chunk column offset
                a, b_, cc, dd = o, o + PW, o + 2 * PW, o + 3 * PW
                u, v = uv[c]
                nc.vector.tensor_tensor(u[:], xt[:, a:a + PW], xt[:, b_:b_ + PW], op=add)
                nc.vector.tensor_tensor(v[:], xt[:, cc:cc + PW], xt[:, dd:dd + PW], op=add)
                init = 0.0 if c == 0 else S[:, o - 1:o]
                # cum -> D slot of S; the scan's op1 folds in the final tree
                # add: state = (u_t + state) + v_t
                nc.vector.tensor_tensor_scan(
                    S[:, dd:dd + PW], u[:], v[:], init, op0=add, op1=add
                )
                # chained reconstruction of the other three phase sums
                nc.vector.tensor_tensor(
                    S[:, cc:cc + PW], S[:, dd:dd + PW], xt[:, dd:dd + PW], op=sub
                )
                nc.vector.tensor_tensor(
                    S[:, b_:b_ + PW], S[:, cc:cc + PW], xt[:, cc:cc + PW], op=sub
                )
                nc.vector.tensor_tensor(
                    S[:, a:a + PW], S[:, b_:b_ + PW], xt[:, b_:b_ + PW], op=sub
                )
                # batched scale via 2x tensor_tensor; output DMAs go on the
                # Scalar queue (idle after C0D0).  The last chunk's multiply
                # is split so its first piece's DMA starts earlier and the
                # final transfer (and thus the completion tail) is short.
                if c < NCH - 1:
                    nc.vector.tensor_tensor(
                        ot[:, o:o + CW], S[:, o:o + CW], rr[:, o:o + CW], op=mult
                    )
                    nc.scalar.dma_start(OUT[:, o:o + CW], ot[:, o:o + CW])
                else:
                    m = o + 1536
                    nc.vector.tensor_tensor(
                        ot[:, o:m], S[:, o:m], rr[:, o:m], op=mult
                    )
                    # split the final transfer across both queues so the
                    # completion tail is half as long
                    h = o + 768
                    nc.sync.dma_start(OUT[:, o:h], ot[:, o:h])
                    nc.scalar.dma_start(OUT[:, h:m], ot[:, h:m])
                    nc.vector.tensor_tensor(
                        ot[:, m:o + CW], S[:, m:o + CW], rr[:, m:o + CW], op=mult
                    )
                    nc.scalar.dma_start(OUT[:, m:o + CW], ot[:, m:o + CW])
    return nc


def _get_nc():
    if "nc" not in _cache:
        _cache["nc"] = _build_nc()
    return _cache["nc"]


def _quad_layout(rowmajor):
    """[rows, L] -> chunk-major quad de-interleave along the last axis:
    out[:, c*2048 + p*512 + k] = in[:, 4*(c*512+k)+p]"""
    r = rowmajor.reshape(-1, NCH, PW, 4)         # [rows, c, k, p]
    return np.ascontiguousarray(r.transpose(0, 1, 3, 2)).reshape(-1, L)


def _quad_unlayout(quad):
    """inverse of _quad_layout"""
    r = quad.reshape(-1, NCH, 4, PW)             # [rows, c, p, k]
    return np.ascontiguousarray(r.transpose(0, 1, 3, 2)).reshape(-1, L)


def _make_in_maps(x):
    idx = np.arange(1, L + 1, dtype=np.float64)
    rrow = (1.0 / idx).astype(np.float16).reshape(1, L)
    rrf = np.ascontiguousarray(np.broadcast_to(_quad_layout(rrow), (P, L)))
    in_maps = []
    shards = []
    for c in range(NCORES):
        b, dh = c // 2, c % 2
        shards.append((b, dh))
        xT = np.ascontiguousarray(x[b, :, dh * P:(dh + 1) * P].T).astype(np.float16)
        in_maps.append({"X": _quad_layout(xT), "RRF": rrf})
    return in_maps, shards


def kernel(x, q):
    from concourse.bass_utils import run_bass_kernel_spmd

    x = np.asarray(x)
    assert x.shape == (B, L, D) and x.dtype == np.float32

    nc = _get_nc()
    in_maps, shards = _make_in_maps(x)
    results = run_bass_kernel_spmd(nc, in_maps, list(range(NCORES))).results

    out = np.empty((B, L, D), dtype=np.float32)
    for c, (b, dh) in enumerate(shards):
        outT = _quad_unlayout(results[c]["OUT"]).astype(np.float32)
        out[b, :, dh * P:(dh + 1) * P] = outT.T
    return out


# revision 18
# speedup vs baseline: 1.1427x; 1.0400x over previous
"""Trainium2 Bass kernel for nn_CausalAttentionPooling.

Math: scores[b,i,j] = x[b,i].q are constant along the softmax axis j, so
softmax over the causal mask yields uniform weights 1/(i+1) on j <= i.
The module is exactly a causal cumulative mean:
    out[b,i,:] = cumsum(x, axis=1)[b,i,:] / (i+1)
(q does not affect the output.)

v2 design (fp16 I/O, quad-decimated scan, DVE-only math):
  - 8 shards = (batch b) x (D-half dh); per core xT = x[b,:,dh*128:+128].T
    as [128(D), 4096(L)] cast to fp16 on host (tolerance is 2e-2; fp16
    roundtrip error ~5e-4).
  - Host de-interleaves L into quad phases, chunk-major:
    X[:, c*2048 + p*512 + k] = xT[:, 4*(c*512+k)+p].  The DVE scan (which
    has no 2x mode and runs at ~2.1 ns/elem) then only processes the
    pairwise-sum sequence y_k = sum_p x_{4k+p} (1024 elems instead of
    4096); everything else runs as 2x-mode fp16 tensor_tensor ops
    (~0.56 ns/elem):
      u = A+B; v = C+D; y = u+v
      cum = scan(y)            -> written into the D slot of S
      s_c = cum - D; s_b = s_c - C; s_a = s_b - B   (chained subs)
      OUT = S * RR             (one batched multiply per chunk)
  - RR = 1/(i+1) in the same chunk-major quad layout, shipped from HBM
    quarter-replicated [32, 4096] and partition-replicated on-chip by
    three SBUF->SBUF DMAs (no PE broadcast, no PSUM).
  - Scan accumulates in fp32 internally regardless of operand dtype.
  - Two chunks (c=0,1) pipeline DMA-in / DVE / DMA-out; chunk 1's scan
    chains via initial=S[:, 2047:2048].
"""

import numpy as np

B, L, D = 4, 4096, 256
NCORES = 8
P = 128            # partitions / D-shard width
NCH = 2            # chunks along L
CW = L // NCH      # columns per chunk (2048)
PW = CW // 4       # phase width within a chunk (512)

_cache = {}


def _split_waits_bir(bir_bytes):
    """This container's walrus build rejects instructions carrying more than
    one (or for some opcodes, two) sync waits.  Hoist multi-wait sync_info
    onto standalone same-engine EventSemaphore instructions inserted
    immediately before the instruction; program order on the engine's stream
    preserves semantics."""
    import orjson

    d = orjson.loads(bir_bytes)
    n = 0
    for fn in d["functions"]:
        for bb in fn["blocks"]:
            out = []
            for inst in bb["instructions"]:
                si = inst.get("sync_info")
                waits = (si or {}).get("on_wait") or []
                if len(waits) > 1:
                    for w in waits:
                        out.append(
                            {
                                "debug": inst.get("debug"),
                                "engine": inst["engine"],
                                "ins": [],
                                "name": f"I-waitfix-{n}",
                                "opcode": "EventSemaphore",
                                "outs": [],
                                "sync_info": {"on_wait": [w], "on_update": []},
                            }
                        )
                        n += 1
                    si["on_wait"] = []
                out.append(inst)
            bb["instructions"] = out
    return orjson.dumps(d)


def _install_bir_patch():
    if _cache.get("patched"):
        return
    import concourse.bass as bass

    orig = bass.Bass.to_json_bytes

    def patched(self):
        return _split_waits_bir(orig(self))

    bass.Bass.to_json_bytes = patched
    _cache["patched"] = True


def _build_nc():
    import concourse.bass as bass
    import concourse.tile as tile
    from concourse import mybir

    _install_bir_patch()

    f16 = mybir.dt.float16
    add = mybir.AluOpType.add
    sub = mybir.AluOpType.subtract
    byp = mybir.AluOpType.bypass
    mult = mybir.AluOpType.mult

    nc = bass.Bass()
    X = nc.declare_dram_parameter("X", [P, L], f16, isOutput=False)
    RRF = nc.declare_dram_parameter("RRF", [P, L], f16, isOutput=False)
    OUT = nc.declare_dram_parameter("OUT", [P, L], f16, isOutput=True)

    with tile.TileContext(nc) as tc:
        with tc.tile_pool(name="sb", bufs=1) as sb:
            xt = sb.tile([P, L], f16, tag="xt")
            rr = sb.tile([P, L], f16, tag="rr")
            S = sb.tile([P, L], f16, tag="S")
            ot = sb.tile([P, L], f16, tag="ot")
            uv = [
                (sb.tile([P, PW], f16, tag=f"u{c}", name=f"u{c}"),
                 sb.tile([P, PW], f16, tag=f"v{c}", name=f"v{c}"))
                for c in range(NCH)
            ]

            # input DMAs: chunk0's tree operands split pairwise across the
            # Sync and Scalar queues so each lands as early as possible
            # (concurrent DMAs round-robin at packet level, so one big DMA
            # finishes late); RRQ + its partition-replication (3 SBUF->SBUF
            # copies) trail on the Scalar queue -- rr is only needed by the
            # first OUT multiply, several microseconds later.
            # rr ships fully replicated from HBM (+1MB read) -- cheaper in
            # latency than on-chip partition replication, whose 2-hop
            # HBM->SBUF->SBUF chain kept gating the first OUT multiply.
            # both chunk0 halves ride the Sync queue back-to-back (the
            # Scalar queue's first transfer starts ~1us later); chunk 1 and
            # rr overlap on the Scalar queue
            # single need-ordered input stream on the Sync queue: each piece
            # lands just before its consumer, and nothing steals HBM
            # bandwidth from the critical path (concurrent queues round-robin
            # at packet granularity).  Outputs get the Scalar queue.
            nc.sync.dma_start(xt[:, 0:1024], X[:, 0:1024])          # A0 B0
            nc.scalar.dma_start(xt[:, 1024:2048], X[:, 1024:2048])  # C0 D0
            nc.sync.dma_start(xt[:, 2048:3072], X[:, 2048:3072])    # A1 B1
            nc.sync.dma_start(xt[:, 3072:4096], X[:, 3072:4096])    # C1 D1
            nc.sync.dma_start(rr[:, 0:2048], RRF[:, 0:2048])
            nc.sync.dma_start(rr[:, 2048:4096], RRF[:, 2048:4096])

            for c in range(NCH):
                o = c * CW          # chunk column offset
                a, b_, cc, dd = o, o + PW, o + 2 * PW, o + 3 * PW
                u, v = uv[c]
                nc.vector.tensor_tensor(u[:], xt[:, a:a + PW], xt[:, b_:b_ + PW], op=add)
                nc.vector.tensor_tensor(v[:], xt[:, cc:cc + PW], xt[:, dd:dd + PW], op=add)
                init = 0.0 if c == 0 else S[:, o - 1:o]
                # cum -> D slot of S; the scan's op1 folds in the final tree
                # add: state = (u_t + state) + v_t
                nc.vector.tensor_tensor_scan(
                    S[:, dd:dd + PW], u[:], v[:], init, op0=add, op1=add
                )
                # chained reconstruction of the other three phase sums
                nc.vector.tensor_tensor(
                    S[:, cc:cc + PW], S[:, dd:dd + PW], xt[:, dd:dd + PW], op=sub
                )
                nc.vector.tensor_tensor(
                    S[:, b_:b_ + PW], S[:, cc:cc + PW], xt[:, cc:cc + PW], op=sub
                )
                nc.vector.tensor_tensor(
                    S[:, a:a + PW], S[:, b_:b_ + PW], xt[:, b_:b_ + PW], op=sub
                )
                # batched scale via 2x tensor_tensor; output DMAs go on the
                # Scalar queue (idle after C0D0).  The last chunk's multiply
                # is split so its first piece's DMA starts earlier and the
                # final transfer (and thus the completion tail) is short.
                if c < NCH - 1:
                    nc.vector.tensor_tensor(
                        ot[:, o:o + CW], S[:, o:o + CW], rr[:, o:o + CW], op=mult
                    )
                    nc.scalar.dma_start(OUT[:, o:o + CW], ot[:, o:o + CW])
                else:
                    m = o + 1536
                    nc.vector.tensor_tensor(
                        ot[:, o:m], S[:, o:m], rr[:, o:m], op=mult
                    )
                    # split the final transfer across both queues so the
                    # completion tail is half as long
                    h = o + 768
                    nc.sync.dma_start(OUT[:, o:h], ot[:, o:h])
                    nc.scalar.dma_start(OUT[:, h:m], ot[:, h:m])
                    nc.vector.tensor_tensor(
                        ot[:, m:o + CW], S[:, m:o + CW], rr[:, m:o + CW], op=mult
                    )
                    nc.scalar.dma_start(OUT[:, m:o + CW], ot[:, m:o + CW])
    return nc


def _get_nc():
    if "nc" not in _cache:
        _cache["nc"] = _build_nc()
    return _cache["nc"]


def _quad_layout(rowmajor):
    """[rows, L] -> chunk-major quad de-interleave along the last axis:
    out[:, c*2048 + p*512 + k] = in[:, 4*(c*512+k)+p]"""
    r = rowmajor.reshape(-1, NCH, PW, 4)         # [rows, c, k, p]
    return np.ascontiguousarray(r.transpose(0, 1, 3, 2)).reshape(-1, L)


def _quad_unlayout(quad):
    """inverse of _quad_layout"""
    r = quad.reshape(-1, NCH, 4, PW)             # [rows, c, p, k]
    return np.ascontiguousarray(r.transpose(0, 1, 3, 2)).reshape(-1, L)


def _make_in_maps(x):
    idx = np.arange(1, L + 1, dtype=np.float64)
    rrow = (1.0 / idx).astype(np.float16).reshape(1, L)
    rrf = np.ascontiguousarray(np.broadcast_to(_quad_layout(rrow), (P, L)))
    in_maps = []
    shards = []
    for c in range(NCORES):
        b, dh = c // 2, c % 2
        shards.append((b, dh))
        xT = np.ascontiguousarray(x[b, :, dh * P:(dh + 1) * P].T).astype(np.float16)
        in_maps.append({"X": _quad_layout(xT), "RRF": rrf})
    return in_maps, shards


def kernel(x, q):
    from concourse.bass_utils import run_bass_kernel_spmd

    x = np.asarray(x)
    assert x.shape == (B, L, D) and x.dtype == np.float32

    nc = _get_nc()
    in_maps, shards = _make_in_maps(x)
    results = run_bass_kernel_spmd(nc, in_maps, list(range(NCORES))).results

    out = np.empty((B, L, D), dtype=np.float32)
    for c, (b, dh) in enumerate(shards):
        outT = _quad_unlayout(results[c]["OUT"]).astype(np.float32)
        out[b, :, dh * P:(dh + 1) * P] = outT.T
    return out
